# revision 1
# baseline (speedup 1.0000x reference)
"""Trainium2 Bass kernel for nn_CausalSelfAttention_14980845928591.

Full-input contract: kernel(**inputs) takes the unsharded numpy inputs and
returns the full [B, T, C] output. Internally shards across 8 NeuronCores:
data-parallel over B (4 batches) x tensor-parallel over heads (2 groups of 8
heads). Causal attention is independent per (batch, head); the output
projection is a partial sum over head groups, reduced on the host.

Device kernel (identical SPMD program, per-core data):
  phase 1: qT/kT (transposed layout), V (natural layout + ones column for
           softmax row-sums), gateT = sigmoid(wg^T @ xT + b) - all via
           float32r matmuls (full PE rate at N>=512, ~1e-4 rel err).
  phase 2: causal attention per head pair, S^T orientation (no transposes):
           S^T tiles = (k-block)^T x q, softmax along the free axis. Softmax without
           max-subtraction (logits are O(5) here, exp is safe in fp32);
           row-sums ride along as a 65th output row via the ones column.
  phase 3: out^T = wp^T @ (O^T * gate^T / rowsum), streamed per i-tile.
"""
import sys

sys.path.insert(0, "/opt/trn_rl_repo")

import numpy as np

import concourse.bass as bass
import concourse.mybir as mybir
import concourse.tile as tile
from concourse import bacc, bass_utils

# Problem shapes (hardcoded per contract).
B, T, C = 4, 2048, 1024
H, D = 16, 64
HL = 8            # heads per core
GC = HL * D       # 512: local q/k/v/gate/proj-row columns
P = 128
KC = C // P       # 8 contraction chunks
NTB = T // 512    # 4 T-blocks of 512
NIT = T // 512    # 4 i-tiles of 512
F32 = mybir.dt.float32
F32R = mybir.dt.float32r

_NC_CACHE = {}


def _build_nc():
    nc = bacc.Bacc("TRN2", target_bir_lowering=False, debug=False, num_devices=8)

    xT = nc.dram_tensor("xT", [C, T], F32R, kind="ExternalInput")
    wqk = nc.dram_tensor("wqk", [C, 2 * GC], F32R, kind="ExternalInput")
    wv = nc.dram_tensor("wv", [C, GC], F32R, kind="ExternalInput")
    wg = nc.dram_tensor("wg", [C, GC], F32R, kind="ExternalInput")
    bg = nc.dram_tensor("bg", [GC], F32, kind="ExternalInput")
    wp = nc.dram_tensor("wp", [GC, C], F32R, kind="ExternalInput")
    outT = nc.dram_tensor("outT", [C, T], F32, kind="ExternalOutput")
    qT_d = nc.dram_tensor("qT_scratch", [GC, T], F32R, kind="Internal")
    r_d = nc.dram_tensor("r_scratch", [NIT, 4, 1024], F32, kind="Internal")

    with tile.TileContext(nc) as tc, \
         tc.tile_pool(name="pers", bufs=1) as pers:
        # ---------- phase-1-scoped pools ----------
        with tc.tile_pool(name="psum_ph1", bufs=4, space="PSUM") as ps1, \
             tc.tile_pool(name="xs", bufs=10) as xs, \
             tc.tile_pool(name="w1", bufs=1) as w1, \
             tc.tile_pool(name="qw", bufs=3) as qwp:

            kT = [pers.tile([P, T], F32R, tag=f"kT{i}", name=f"kT{i}") for i in range(4)]
            vsb = [pers.tile([P, HL, 65], F32R, tag=f"v{j}", name=f"v{j}") for j in range(T // P)]
            gT = [pers.tile([P, T], F32, tag=f"gT{i}", name=f"gT{i}") for i in range(4)]
            wpsb = [pers.tile([P, C], F32R, tag=f"wp{k}", name=f"wp{k}") for k in range(4)]
            umask = pers.tile([P, P], F32R, tag="umask")
            zbias = pers.tile([P, 1], F32, tag="zbias")
            nc.gpsimd.memset(zbias, 0.0)
            bgsb = pers.tile([P, 4], F32, tag="bg")

            # constants
            nc.gpsimd.memset(umask.bitcast(F32), 1.0)
            # keep element where (col - row) >= 0  (upper triangular incl diag)
            nc.gpsimd.affine_select(
                out=umask.bitcast(F32), in_=umask.bitcast(F32), compare_op=mybir.AluOpType.is_ge,
                fill=0.0, base=0, pattern=[[1, P]], channel_multiplier=-1,
            )
            nc.sync.dma_start(out=bgsb, in_=bg.rearrange("(m p) -> p m", m=4))

            # weights
            wqksb = [w1.tile([P, 2 * GC], F32R, tag=f"wqk{k}", name=f"wqk{k}") for k in range(KC)]
            wvsb = [w1.tile([P, GC], F32R, tag=f"wv{k}", name=f"wv{k}") for k in range(KC)]
            wgsb = [w1.tile([P, GC], F32R, tag=f"wg{k}", name=f"wg{k}") for k in range(KC)]
            # interleave wqk with the first T-block of xT so the PE can start
            # the k-accumulation as soon as each pair of tiles lands
            xt0 = []
            for k in range(KC):
                nc.sync.dma_start(out=wqksb[k], in_=wqk[k * P:(k + 1) * P, :])
                t_ = xs.tile([P, 512], F32R, tag="xs", name="xs")
                nc.sync.dma_start(out=t_, in_=xT[k * P:(k + 1) * P, 0:512])
                xt0.append(t_)

            # ---------- phase 1: projections, per T-block ----------
            for tb in range(NTB):
                tsl = slice(tb * 512, (tb + 1) * 512)
                if tb == 0:
                    xt = xt0
                else:
                    xt = []
                    for k in range(KC):
                        t_ = xs.tile([P, 512], F32R, tag="xs", name="xs")
                        nc.sync.dma_start(out=t_, in_=xT[k * P:(k + 1) * P, tsl])
                        xt.append(t_)
                # q (m 0..3, spilled) and k (m 4..7, resident)
                for m in range(8):
                    ps = ps1.tile([P, 512], F32, tag="ps1", name="ps1")
                    for k in range(KC):
                        nc.tensor.matmul(
                            ps, wqksb[k][:, m * P:(m + 1) * P], xt[k],
                            start=(k == 0), stop=(k == KC - 1))
                    if m < 4:
                        q_ = qwp.tile([P, 512], F32R, tag="qw", name="qw")
                        nc.vector.tensor_copy(out=q_, in_=ps)
                        nc.sync.dma_start(out=qT_d[m * P:(m + 1) * P, tsl], in_=q_)
                    else:
                        nc.vector.tensor_copy(out=kT[m - 4][:, tsl], in_=ps)
                if tb == 0:
                    for k in range(KC):
                        nc.sync.dma_start(out=wvsb[k], in_=wv[k * P:(k + 1) * P, :])
                        nc.sync.dma_start(out=wgsb[k], in_=wg[k * P:(k + 1) * P, :])
                # gate (sigmoid fused on ACT)
                for m in range(4):
                    ps = ps1.tile([P, 512], F32, tag="ps1", name="ps1")
                    for k in range(KC):
                        nc.tensor.matmul(
                            ps, wgsb[k][:, m * P:(m + 1) * P], xt[k],
                            start=(k == 0), stop=(k == KC - 1))
                    nc.scalar.activation(
                        gT[m][:, tsl], ps, mybir.ActivationFunctionType.Sigmoid,
                        bias=bgsb[:, m:m + 1], scale=1.0)
                # V natural (+ones column)
                for mt in range(4):
                    j = tb * 4 + mt
                    ps = ps1.tile([P, 512], F32, tag="ps1", name="ps1")
                    for k in range(KC):
                        nc.tensor.matmul(
                            ps, xt[k][:, mt * P:(mt + 1) * P], wvsb[k],
                            start=(k == 0), stop=(k == KC - 1))
                    nc.vector.tensor_copy(
                        out=vsb[j][:, :, 0:64],
                        in_=ps.rearrange("p (h d) -> p h d", h=HL))
                    nc.vector.memset(vsb[j][:, :, 64:65].bitcast(F32), 1.0)

        for k in range(4):
            nc.sync.dma_start(out=wpsb[k], in_=wp[k * P:(k + 1) * P, :])

        # ---------- phase 2+3: attention + output projection ----------
        with tc.tile_pool(name="qs", bufs=2) as qsp, \
             tc.tile_pool(name="pT", bufs=3) as pTp, \
             tc.tile_pool(name="osb", bufs=1) as osbp, \
             tc.tile_pool(name="rr", bufs=1) as rrp, \
             tc.tile_pool(name="rbt", bufs=3) as rbtp, \
             tc.tile_pool(name="yT", bufs=2) as yTp, \
             tc.tile_pool(name="ob", bufs=4) as obp, \
             tc.tile_pool(name="ps_s", bufs=2, space="PSUM") as pss, \
             tc.tile_pool(name="ps_o", bufs=1, space="PSUM") as pso, \
             tc.tile_pool(name="ps_p", bufs=2, space="PSUM") as psp:

            for it in range(NIT):
                isl = slice(it * 512, (it + 1) * 512)
                qs = []
                for p in range(4):
                    q_ = qsp.tile([P, 512], F32R, tag=f"q{p}", name=f"q{p}")
                    nc.sync.dma_start(out=q_, in_=qT_d[p * P:(p + 1) * P, isl])
                    qs.append(q_)
                r8 = rrp.tile([P, 1024], F32, tag="r8", name="r8")
                nc.vector.memset(r8, 1.0)
                osb = []
                njb = 4 * it + 4
                for p in range(4):
                    O2 = pso.tile([65, 1024], F32, tag="O2", name="O2")
                    for jb in range(njb):
                        jsl = slice(jb * P, (jb + 1) * P)
                        s2 = pss.tile([P, 1024], F32, tag="s2", name="s2")
                        # columns below the causal diagonal block are never
                        # consumed: restrict diagonal-band matmuls/exp to the
                        # live column range [c0:512) of each head's half.
                        ko = jb - 4 * it
                        c0 = P * ko if ko > 0 else 0
                        nc.tensor.matmul(s2[:, c0:512], kT[p][0:64, jsl],
                                         qs[p][0:64, c0:512],
                                         start=True, stop=True)
                        nc.tensor.matmul(s2[:, 512 + c0:1024],
                                         kT[p][64:128, jsl],
                                         qs[p][64:128, c0:512],
                                         start=True, stop=True)
                        pT = pTp.tile([P, 1024], F32R, tag="pT", name="pT")
                        if ko > 0:
                            nc.scalar.activation(
                                pT[:, c0:512], s2[:, c0:512],
                                mybir.ActivationFunctionType.Exp,
                                bias=zbias, scale=0.125)
                            nc.scalar.activation(
                                pT[:, 512 + c0:1024], s2[:, 512 + c0:1024],
                                mybir.ActivationFunctionType.Exp,
                                bias=zbias, scale=0.125)
                        else:
                            nc.scalar.activation(
                                pT, s2, mybir.ActivationFunctionType.Exp,
                                bias=zbias, scale=0.125)
                        if ko >= 0:
                            nc.vector.tensor_mul(pT[:, c0:c0 + P],
                                                 pT[:, c0:c0 + P], umask)
                            nc.vector.tensor_mul(pT[:, 512 + c0:512 + c0 + P],
                                                 pT[:, 512 + c0:512 + c0 + P],
                                                 umask)
                        st, sp = (jb == 0), (jb == njb - 1)
                        nc.tensor.matmul(O2[:, c0:512], vsb[jb][:, 2 * p, :],
                                         pT[:, c0:512], start=st, stop=sp,
                                         skip_group_check=True)
                        nc.tensor.matmul(O2[:, 512 + c0:1024],
                                         vsb[jb][:, 2 * p + 1, :],
                                         pT[:, 512 + c0:1024],
                                         start=st, stop=sp,
                                         skip_group_check=True)
                    o_ = osbp.tile([P, 512], F32, tag=f"o{p}", name=f"o{p}")
                    nc.vector.tensor_copy(out=o_[0:64, :], in_=O2[0:64, 0:512])
                    nc.vector.tensor_copy(out=o_[64:128, :], in_=O2[0:64, 512:1024])
                    nc.vector.tensor_copy(out=r8[32 * p:32 * p + 1, :],
                                           in_=O2[64:65, :])
                    osb.append(o_)
                # reciprocal of row sums via exp(-ln(r)) on ACT; only rows
                # 0/32/64/96 hold data, the other partitions compute junk.
                r8ln = rrp.tile([P, 1024], F32, tag="r8ln", name="r8ln")
                r8rec = rrp.tile([P, 1024], F32, tag="r8rec", name="r8rec")
                nc.scalar.activation(r8ln, r8,
                                     mybir.ActivationFunctionType.Ln,
                                     bias=zbias)
                nc.scalar.activation(r8rec, r8ln,
                                     mybir.ActivationFunctionType.Exp,
                                     bias=zbias, scale=-1.0)
                for p in range(4):
                    nc.sync.dma_start(out=r_d[it, p],
                                      in_=r8rec[32 * p:32 * p + 1, :])
                # gate * recip, build yT (f32r) for the projection
                yT = [yTp.tile([P, 512], F32R, tag=f"y{p}", name=f"y{p}") for p in range(4)]
                for p in range(4):
                    rb = rbtp.tile([P, 512], F32, tag="rb", name="rb")
                    for half in range(2):
                        rsrc = r_d[it, p:p + 1, half * 512:(half + 1) * 512]
                        nc.sync.dma_start(
                            out=rb[half * 64:(half + 1) * 64, :],
                            in_=rsrc.to_broadcast((64, 512)))
                    t_ = rbtp.tile([P, 512], F32, tag="t", name="t")
                    nc.vector.tensor_mul(t_, osb[p], gT[p][:, isl])
                    nc.vector.tensor_mul(yT[p], t_, rb)
                # output projection for this i-tile
                for m in range(8):
                    ps = psp.tile([P, 512], F32, tag="po", name="po")
                    for k in range(4):
                        nc.tensor.matmul(
                            ps, wpsb[k][:, m * P:(m + 1) * P], yT[k],
                            start=(k == 0), stop=(k == 3))
                    ob = obp.tile([P, 512], F32, tag="ob", name="ob")
                    nc.vector.tensor_copy(out=ob, in_=ps)
                    nc.sync.dma_start(out=outT[m * P:(m + 1) * P, isl], in_=ob)

    nc.compile()
    return nc


def kernel(x, w_attn, w_proj, w_gate, b_gate):
    x = np.ascontiguousarray(np.asarray(x, dtype=np.float32))
    w_attn = np.asarray(w_attn, dtype=np.float32)
    w_proj = np.asarray(w_proj, dtype=np.float32)
    w_gate = np.asarray(w_gate, dtype=np.float32)
    b_gate = np.asarray(b_gate, dtype=np.float32)

    if "nc" not in _NC_CACHE:
        _NC_CACHE["nc"] = _build_nc()
    nc = _NC_CACHE["nc"]

    in_maps = []
    for c in range(8):
        b, g = c // 2, c % 2
        hsl = slice(g * GC, (g + 1) * GC)
        in_maps.append({
            "xT": np.ascontiguousarray(x[b].T),
            "wqk": np.ascontiguousarray(
                np.concatenate([w_attn[:, hsl], w_attn[:, C + g * GC:C + (g + 1) * GC]],
                               axis=1)),
            "wv": np.ascontiguousarray(w_attn[:, 2 * C + g * GC:2 * C + (g + 1) * GC]),
            "wg": np.ascontiguousarray(w_gate[:, hsl]),
            "bg": np.ascontiguousarray(b_gate[hsl]),
            "wp": np.ascontiguousarray(w_proj[hsl, :]),
        })

    res = bass_utils.run_bass_kernel_spmd(nc, in_maps, core_ids=list(range(8)))

    out = np.empty((B, T, C), dtype=np.float32)
    for b in range(B):
        acc = res.results[2 * b]["outT"].astype(np.float32)
        acc = acc + res.results[2 * b + 1]["outT"]
        out[b] = acc.T
    return out



# revision 2
# speedup vs baseline: 1.3735x; 1.3735x over previous
"""Trainium2 Bass kernel for nn_CausalSelfAttention_14980845928591.

Full-input contract: kernel(**inputs) takes the unsharded numpy inputs and
returns the full [B, T, C] output. Internally shards across 8 NeuronCores:
data-parallel over B (4 batches) x tensor-parallel over heads (2 groups of 8
heads). The output projection is a partial sum over head groups, reduced on
the host.

Single merged pipeline per core (no phase barriers):
  - projections (q/k/v/gate) per 512-wide T-block in fp32r (K=128 matmuls
    stream fastest in fp32r; weight load hides internally),
  - causal attention per i-tile in fp16 (K=64 / M=65 matmuls hit full PE
    stream rate in 16-bit; fp16 keeps quantization error ~1e-3),
  - output projection in fp32r.
Emission interleaves attention jb-steps (primary) with projection tile-jobs
(secondary) so the PE queue always holds independent work behind the
exp-dependent AV matmuls. exp runs on Act; sigmoid is computed as
0.5*(1+tanh(x/2)) so Act only ever needs the exp/tanh table (no reloads);
the 0.5 is folded into host-scaled w_proj and the +1 into a fused
scalar_tensor_tensor on DVE. Softmax row sums ride as a 65th V row; the
reciprocal uses the fast DVE approximation and is broadcast across
partitions via a DRAM round trip, all software-pipelined one i-tile behind
attention.
"""
import sys

sys.path.insert(0, "/opt/trn_rl_repo")

import numpy as np

import concourse.bass as bass
import concourse.mybir as mybir
import concourse.tile as tile
from concourse import bacc, bass_utils

# Problem shapes (hardcoded per contract).
B, T, C = 4, 2048, 1024
H, D = 16, 64
HL = 8            # heads per core
GC = HL * D       # 512: local q/k/v/gate/proj-row columns
P = 128
KC = C // P       # 8 contraction chunks
NTB = T // 512    # 4 T-blocks of 512
F32 = mybir.dt.float32
F32R = mybir.dt.float32r
F16 = mybir.dt.float16

_NC_CACHE = {}


def _build_nc():
    nc = bacc.Bacc("TRN2", target_bir_lowering=False, debug=False, num_devices=8)

    xT = nc.dram_tensor("xT", [C, T], F32R, kind="ExternalInput")
    wqk = nc.dram_tensor("wqk", [C, 2 * GC], F32R, kind="ExternalInput")
    wv = nc.dram_tensor("wv", [C, GC], F32R, kind="ExternalInput")
    wg = nc.dram_tensor("wg", [C, GC], F32R, kind="ExternalInput")
    bgh = nc.dram_tensor("bgh", [GC], F32, kind="ExternalInput")   # 0.5*b_gate
    wp = nc.dram_tensor("wp", [GC, C], F32R, kind="ExternalInput") # 0.5*w_proj
    outT = nc.dram_tensor("outT", [C, T], F32, kind="ExternalOutput")
    r_d = nc.dram_tensor("r_scratch", [NTB, 4, 1024], F32, kind="Internal")

    EXP = mybir.ActivationFunctionType.Exp
    TANH = mybir.ActivationFunctionType.Tanh

    with tile.TileContext(nc) as tc, \
         tc.tile_pool(name="pers", bufs=1) as pers, \
         tc.tile_pool(name="rot", bufs=2) as rot, \
         tc.tile_pool(name="psp", bufs=1, space="PSUM") as psp:

        # ---------- persistent tiles ----------
        kT = [pers.tile([P, T], F16, tag=f"kT{i}", name=f"kT{i}") for i in range(4)]
        vsb = [pers.tile([P, HL, 65], F16, tag=f"v{j}", name=f"v{j}")
               for j in range(T // P)]
        wqksb = [pers.tile([P, 2 * GC], F32R, tag=f"wqk{k}", name=f"wqk{k}")
                 for k in range(KC)]
        wvsb = [pers.tile([P, GC], F32R, tag=f"wv{k}", name=f"wv{k}")
                for k in range(KC)]
        wgsb = [pers.tile([P, GC], F32R, tag=f"wg{k}", name=f"wg{k}")
                for k in range(KC)]
        wpsb = [pers.tile([P, C], F32R, tag=f"wp{k}", name=f"wp{k}")
                for k in range(4)]
        umaskf = pers.tile([P, P], F32, tag="umaskf")
        umask = pers.tile([P, P], F16, tag="umask")
        zbias = pers.tile([P, 1], F32, tag="zbias")
        bgsb = pers.tile([P, 4], F32, tag="bg")

        # constants
        nc.gpsimd.memset(zbias, 0.0)
        nc.gpsimd.memset(umaskf, 1.0)
        # keep element where (col - row) >= 0 (upper triangular incl diag)
        nc.gpsimd.affine_select(
            out=umaskf, in_=umaskf, compare_op=mybir.AluOpType.is_ge,
            fill=0.0, base=0, pattern=[[1, P]], channel_multiplier=-1,
        )
        nc.vector.tensor_copy(out=umask, in_=umaskf)
        nc.sync.dma_start(out=bgsb, in_=bgh.rearrange("(m p) -> p m", m=4))

        state = {}

        # ---------- startup: wqk + x(tb0) interleaved ----------
        xt0 = []
        for k in range(KC):
            nc.sync.dma_start(out=wqksb[k], in_=wqk[k * P:(k + 1) * P, :])
            t_ = rot.tile([P, 512], F32R, tag="xs", name="xs", bufs=10)
            nc.sync.dma_start(out=t_, in_=xT[k * P:(k + 1) * P, 0:512])
            xt0.append(t_)
        state[("xt", 0)] = xt0

        # ---------- job builders ----------
        def mk_xt_dma(tb):
            def job():
                xt = []
                for k in range(KC):
                    t_ = rot.tile([P, 512], F32R, tag="xs", name="xs", bufs=10)
                    nc.sync.dma_start(
                        out=t_, in_=xT[k * P:(k + 1) * P, tb * 512:(tb + 1) * 512])
                    xt.append(t_)
                state[("xt", tb)] = xt
            return job

        def mk_wvg_dma():
            def job():
                for k in range(KC):
                    nc.sync.dma_start(out=wvsb[k], in_=wv[k * P:(k + 1) * P, :])
                    nc.sync.dma_start(out=wgsb[k], in_=wg[k * P:(k + 1) * P, :])
            return job

        def mk_wp_dma():
            def job():
                for k in range(4):
                    nc.sync.dma_start(out=wpsb[k], in_=wp[k * P:(k + 1) * P, :])
            return job

        def mk_qk(tb, m):
            def job():
                xt = state[("xt", tb)]
                tsl = slice(tb * 512, (tb + 1) * 512)
                ps = psp.tile([P, 512], F32, tag="psA", name="psA", bufs=2)
                for k in range(KC):
                    nc.tensor.matmul(ps, wqksb[k][:, m * P:(m + 1) * P], xt[k],
                                     start=(k == 0), stop=(k == KC - 1))
                if m < 4:
                    q_t = rot.tile([P, 512], F16, tag=f"q{m}", name=f"q{m}", bufs=2)
                    nc.vector.tensor_copy(out=q_t, in_=ps)
                    state[("q", tb, m)] = q_t
                else:
                    nc.vector.tensor_copy(out=kT[m - 4][:, tsl], in_=ps)
            return job

        def mk_gate(tb, m):
            def job():
                xt = state[("xt", tb)]
                ps = psp.tile([P, 512], F32, tag="psA", name="psA", bufs=2)
                for k in range(KC):
                    nc.tensor.matmul(ps, wgsb[k][:, m * P:(m + 1) * P], xt[k],
                                     start=(k == 0), stop=(k == KC - 1))
                g_t = rot.tile([P, 512], F16, tag=f"g{m}", name=f"g{m}", bufs=2)
                # tanh(0.5*x + 0.5*b); sigmoid = 0.5*(1+tanh(...)): the +1 is
                # fused into the gate multiply, the 0.5 into host-scaled wp.
                nc.scalar.activation(g_t, ps, TANH, bias=bgsb[:, m:m + 1],
                                     scale=0.5)
                state[("g", tb, m)] = g_t
            return job

        def mk_v(tb, mt):
            def job():
                xt = state[("xt", tb)]
                j = tb * 4 + mt
                ps = psp.tile([P, 512], F32, tag="psA", name="psA", bufs=2)
                for k in range(KC):
                    nc.tensor.matmul(ps, xt[k][:, mt * P:(mt + 1) * P], wvsb[k],
                                     start=(k == 0), stop=(k == KC - 1))
                nc.vector.tensor_copy(
                    out=vsb[j][:, :, 0:64],
                    in_=ps.rearrange("p (h d) -> p h d", h=HL))
                nc.vector.memset(vsb[j][:, :, 64:65], 1.0)
            return job

        def p1_jobs(tb, with_xt=True):
            jobs = []
            if with_xt:
                jobs.append(mk_xt_dma(tb))
            jobs += [mk_qk(tb, m) for m in range(8)]
            if tb == 0:
                jobs.append(mk_wvg_dma())
            jobs += [mk_v(tb, mt) for mt in range(4)]
            jobs += [mk_gate(tb, m) for m in range(4)]
            return jobs

        # ---------- attention ----------
        def emit_av(it, p, jb, njb):
            ko = jb - 4 * it
            c0 = P * ko if ko > 0 else 0
            pT = state.pop(("pT", it, p, jb))
            st, sp = (jb == 0), (jb == njb - 1)
            if st:
                state[("O2", it, p)] = psp.tile([65, 1024], F32, tag="O2",
                                                name="O2", bufs=1)
            O2 = state[("O2", it, p)]
            nc.tensor.matmul(O2[:, c0:512], vsb[jb][:, 2 * p, :],
                             pT[:, c0:512], start=st, stop=sp,
                             skip_group_check=True)
            nc.tensor.matmul(O2[:, 512 + c0:1024], vsb[jb][:, 2 * p + 1, :],
                             pT[:, 512 + c0:1024], start=st, stop=sp,
                             skip_group_check=True)

        def mk_jb(it, p, jb, njb):
            def job():
                ko = jb - 4 * it
                c0 = P * ko if ko > 0 else 0
                jsl = slice(jb * P, (jb + 1) * P)
                q_t = state[("q", it, p)]
                s2 = psp.tile([P, 1024], F32, tag="s2", name="s2", bufs=2)
                nc.tensor.matmul(s2[:, c0:512], kT[p][0:64, jsl],
                                 q_t[0:64, c0:512], start=True, stop=True)
                nc.tensor.matmul(s2[:, 512 + c0:1024], kT[p][64:128, jsl],
                                 q_t[64:128, c0:512], start=True, stop=True)
                pT = rot.tile([P, 1024], F16, tag="pT", name="pT", bufs=3)
                if ko > 0:
                    nc.scalar.activation(pT[:, c0:512], s2[:, c0:512], EXP,
                                         bias=zbias, scale=0.125)
                    nc.scalar.activation(pT[:, 512 + c0:1024],
                                         s2[:, 512 + c0:1024], EXP,
                                         bias=zbias, scale=0.125)
                else:
                    nc.scalar.activation(pT, s2, EXP, bias=zbias, scale=0.125)
                if ko >= 0:
                    # causal mask on the diagonal 128x128 block (Pool engine)
                    nc.gpsimd.tensor_mul(pT[:, c0:c0 + P], pT[:, c0:c0 + P],
                                         umask)
                    nc.gpsimd.tensor_mul(pT[:, 512 + c0:512 + c0 + P],
                                         pT[:, 512 + c0:512 + c0 + P], umask)
                state[("pT", it, p, jb)] = pT
                if jb > 0:
                    emit_av(it, p, jb - 1, njb)
            return job

        def mk_r8(it):
            def job():
                r8_t = rot.tile([P, 1024], F32, tag="r8", name="r8", bufs=2)
                nc.vector.memset(r8_t, 1.0)
                state[("r8", it)] = r8_t
            return job

        def mk_tail(it, p, njb):
            def job():
                emit_av(it, p, njb - 1, njb)
                O2 = state.pop(("O2", it, p))
                o_t = rot.tile([P, 512], F16, tag=f"o{p}", name=f"o{p}", bufs=2)
                nc.vector.tensor_copy(out=o_t[0:64, :], in_=O2[0:64, 0:512])
                nc.vector.tensor_copy(out=o_t[64:128, :], in_=O2[0:64, 512:1024])
                r8_t = state[("r8", it)]
                nc.vector.tensor_copy(out=r8_t[32 * p:32 * p + 1, :],
                                      in_=O2[64:65, :])
                state[("osb", it, p)] = o_t
            return job

        def attn_jobs(it):
            njb = 4 * it + 4
            jobs = [mk_r8(it)]
            for p in range(4):
                jobs += [mk_jb(it, p, jb, njb) for jb in range(njb)]
                jobs.append(mk_tail(it, p, njb))
            return jobs

        # ---------- normalization + output projection ----------
        def mk_recip(it):
            def job():
                r8_t = state.pop(("r8", it))
                rr = rot.tile([P, 1024], F32, tag="rrec", name="rrec", bufs=2)
                nc.vector.reciprocal_approx_fast(out=rr, in_=r8_t)
                for p in range(4):
                    nc.sync.dma_start(out=r_d[it, p],
                                      in_=rr[32 * p:32 * p + 1, :])
            return job

        def mk_rb(it):
            def job():
                for p in range(4):
                    rb_t = rot.tile([P, 512], F32, tag=f"rb{p}", name=f"rb{p}",
                                    bufs=1)
                    for half in range(2):
                        rsrc = r_d[it, p:p + 1, half * 512:(half + 1) * 512]
                        nc.sync.dma_start(
                            out=rb_t[half * 64:(half + 1) * 64, :],
                            in_=rsrc.to_broadcast((64, 512)))
                    state[("rb", it, p)] = rb_t
            return job

        def mk_y(it, p):
            def job():
                g_t = state.pop(("g", it, p))
                o_t = state.pop(("osb", it, p))
                rb_t = state.pop(("rb", it, p))
                tt = rot.tile([P, 512], F32, tag="tt", name="tt", bufs=2)
                # (1 + tanh) * O  -- the 0.5 of the sigmoid identity is folded
                # into wp (host-scaled), the row-sum recip comes via rb.
                nc.vector.scalar_tensor_tensor(
                    out=tt, in0=g_t, scalar=1.0, in1=o_t,
                    op0=mybir.AluOpType.add, op1=mybir.AluOpType.mult)
                y_t = rot.tile([P, 512], F32R, tag=f"y{p}", name=f"y{p}", bufs=1)
                nc.vector.tensor_mul(y_t, tt, rb_t)
                state[("y", it, p)] = y_t
            return job

        def norm_jobs(it):
            return [mk_recip(it), mk_rb(it)] + [mk_y(it, p) for p in range(4)]

        def mk_proj(it, m):
            def job():
                ps = psp.tile([P, 512], F32, tag="psA", name="psA", bufs=2)
                for k in range(4):
                    nc.tensor.matmul(ps, wpsb[k][:, m * P:(m + 1) * P],
                                     state[("y", it, k)],
                                     start=(k == 0), stop=(k == 3))
                ob = rot.tile([P, 512], F32, tag="ob", name="ob", bufs=3)
                nc.vector.tensor_copy(out=ob, in_=ps)
                nc.sync.dma_start(
                    out=outT[m * P:(m + 1) * P, it * 512:(it + 1) * 512],
                    in_=ob)
                if m == 7:
                    for k in range(4):
                        state.pop(("y", it, k))
            return job

        def proj_jobs(it):
            return [mk_proj(it, m) for m in range(8)]

        # ---------- emission schedule ----------
        def emit_interleaved(primary, secondary):
            np_, ns = len(primary), len(secondary)
            si = 0
            for i, job in enumerate(primary):
                job()
                target = ((i + 1) * ns) // np_
                while si < target:
                    secondary[si]()
                    si += 1
            while si < ns:
                secondary[si]()
                si += 1

        for job in p1_jobs(0, with_xt=False):
            job()
        for it in range(NTB):
            primary = attn_jobs(it)
            secondary = []
            if it == 0:
                secondary.append(mk_wp_dma())
            if it >= 1:
                secondary += norm_jobs(it - 1)
                secondary += proj_jobs(it - 1)
            if it + 1 < NTB:
                secondary += p1_jobs(it + 1)
            emit_interleaved(primary, secondary)
        for job in norm_jobs(NTB - 1) + proj_jobs(NTB - 1):
            job()

    nc.compile()
    return nc


def kernel(x, w_attn, w_proj, w_gate, b_gate):
    x = np.ascontiguousarray(np.asarray(x, dtype=np.float32))
    w_attn = np.asarray(w_attn, dtype=np.float32)
    w_proj = np.asarray(w_proj, dtype=np.float32)
    w_gate = np.asarray(w_gate, dtype=np.float32)
    b_gate = np.asarray(b_gate, dtype=np.float32)

    if "nc" not in _NC_CACHE:
        _NC_CACHE["nc"] = _build_nc()
    nc = _NC_CACHE["nc"]

    in_maps = []
    for c in range(8):
        b, g = c // 2, c % 2
        hsl = slice(g * GC, (g + 1) * GC)
        in_maps.append({
            "xT": np.ascontiguousarray(x[b].T),
            "wqk": np.ascontiguousarray(
                np.concatenate([w_attn[:, hsl],
                                w_attn[:, C + g * GC:C + (g + 1) * GC]],
                               axis=1)),
            "wv": np.ascontiguousarray(w_attn[:, 2 * C + g * GC:2 * C + (g + 1) * GC]),
            "wg": np.ascontiguousarray(w_gate[:, hsl]),
            "bgh": np.ascontiguousarray(b_gate[hsl] * 0.5),
            "wp": np.ascontiguousarray(w_proj[hsl, :] * 0.5),
        })

    res = bass_utils.run_bass_kernel_spmd(nc, in_maps, core_ids=list(range(8)))

    out = np.empty((B, T, C), dtype=np.float32)
    for b in range(B):
        acc = res.results[2 * b]["outT"].astype(np.float32)
        acc = acc + res.results[2 * b + 1]["outT"]
        out[b] = acc.T
    return out
